# revision 51
# baseline (speedup 1.0000x reference)
"""BiAttention Trainium2 kernel (8 NeuronCores) with Cmask row compaction.

Masked C rows (Cmask==0) produce exactly -1e30 regardless of compute, and
rows are independent after the softmax over Lq. So: on the host, gather
only the unmasked rows, pack them into 16 "slots" (2 per core, each slot
bound to one batch's Q), run the dense pipeline on ~9 tiles/slot instead
of 16, and scatter results back (masked rows filled with -1e30 host-side).

Per slot (one batch b, up to nct*128 compacted rows):
  C_ = lrelu(C @ W1.T + b1); Q_ = lrelu(Q @ W1.T + b1)     [fp32r matmuls]
  S  = C_ @ Q_.T, masked (Qmask cols -> -1e30), softmax over Lq
  A  = S_ @ Q
  cat = [C, A, A-C, A*C]
  fuse = tanh(cat @ Wf.T + bf); gate = sigmoid(cat @ Wg.T + bg)   [bf16]
  out = gate*fuse + (1-gate)*cat

The 4-block cat GEMM is folded into 3 blocks on the host:
  cat @ W.T = C @ (W0-W2).T + A @ (W1+W2).T + (A*C) @ W3.T
where Wk = W[:, k*512:(k+1)*512].

sigmoid(x) = 0.5*tanh(x/2)+0.5 so every ACT function used (Identity/Copy/
Exp/Tanh) lives in one table set ("exp_and_others") -- no table reloads.

Emission is software-pipelined: each c-tile's attention chain (4 parts:
S+softmax, P^T, attn, attn^T+cat blocks) and each group's C^T/C_^T
prologue are emitted in slots between earlier tiles' GEMM psum groups,
so the PE queue never drains waiting on the scalar/vector softmax chain.
"""
import sys

sys.path.insert(0, "/opt/trn_rl_repo")

import math
from collections import deque
from contextlib import ExitStack

import numpy as np
import ml_dtypes

import concourse.bass as bass
import concourse.tile as tile
from concourse import bacc, mybir
from concourse.bass_utils import run_bass_kernel_spmd
from concourse.masks import make_identity

F32 = mybir.dt.float32
F32R = mybir.dt.float32r
BF16 = mybir.dt.bfloat16
F8 = mybir.dt.float8e4
AF = mybir.ActivationFunctionType
ALU = mybir.AluOpType
AX = mybir.AxisListType
DR = mybir.MatmulPerfMode.DoubleRow
BF16_NP = ml_dtypes.bfloat16
F8_NP = ml_dtypes.float8_e4m3

# The A*C cat block runs through the fused GEMM in fp8e4 (DoubleRow);
# all GEMM weights are pre-scaled by WSCALE on the host (fp8 needs it to
# clear the subnormal range) and the epilogue activations divide it out.
WSCALE = 64.0

N_CORES = 8
NSLOTS = 2                      # slots per core
NSLOT_TOTAL = N_CORES * NSLOTS
B_FULL, LC_FULL, LQ, D = 16, 2048, 512, 512


def _f32(ap):
    return ap.bitcast(F32)


def _groups(nct):
    """Split nct tiles into groups of 4/3 (last may be smaller)."""
    gs, rem = [], nct
    while rem > 4:
        if rem % 4 == 0:
            gs.append(4)
            rem -= 4
        else:
            gs.append(3)
            rem -= 3
    gs.append(rem)
    return gs


def _body(ctx: ExitStack, tc, io, caps, gemm_bias, repeat=1,
          xslot=2, use_fp8=True):
    """caps: tiles per slot, e.g. (9, 8) -- slot s of every core runs
    caps[s] c-tiles. Per-slot group tables are indexed by slot id."""
    nc = tc.nc
    nslots = len(caps)
    groups_s = [_groups(c) for c in caps]
    ngrp_s = [len(g) for g in groups_s]
    base_s = [[sum(gs[:g]) for g in range(len(gs))] for gs in groups_s]
    grp_of_s = [[g for g in range(len(gs)) for _ in range(gs[g])]
                for gs in groups_s]

    cons = ctx.enter_context(tc.tile_pool(name="cons", bufs=1))
    batch = ctx.enter_context(tc.tile_pool(name="batch", bufs=1))
    grp = ctx.enter_context(tc.tile_pool(name="grp", bufs=1))
    grp2 = ctx.enter_context(tc.tile_pool(name="grp2", bufs=2))
    cn_pool = ctx.enter_context(tc.tile_pool(name="cn", bufs=4))
    chain = ctx.enter_context(tc.tile_pool(name="chain", bufs=1))
    psA = ctx.enter_context(tc.tile_pool(name="psA", bufs=2, space="PSUM"))
    psB = ctx.enter_context(tc.tile_pool(name="psB", bufs=2, space="PSUM"))
    psG = ctx.enter_context(tc.tile_pool(name="psG", bufs=4, space="PSUM"))

    # ---- weights: DMAs drip-fed at critical emission points (b==0 only) ----
    wf_ch, wg_ch = [], []
    pending_w = []
    nbf = 8 if use_fp8 else 12
    for jc in range(nbf):
        wf = cons.tile([128, 2048], BF16, tag=f"wft{jc}")
        wg = cons.tile([128, 2048], BF16, tag=f"wgt{jc}")
        pending_w.append((wf[:], io["wft"][:, jc * 2048:(jc + 1) * 2048]))
        pending_w.append((wg[:], io["wgt"][:, jc * 2048:(jc + 1) * 2048]))
        wf_ch.append(wf)
        wg_ch.append(wg)
    if use_fp8:
        wf8 = cons.tile([128, 4 * 2048], F8, tag="wf8")
        wg8 = cons.tile([128, 4 * 2048], F8, tag="wg8")
        for half in range(2):
            sl = slice(half * 4096, (half + 1) * 4096)
            pending_w.append((wf8[:, sl], io["wf8"][:, sl]))
            pending_w.append((wg8[:, sl], io["wg8"][:, sl]))

    def drip_w(npairs):
        for _ in range(2 * npairs):
            if pending_w:
                t, src = pending_w.pop(0)
                nc.sync.dma_start(t, src)

    w1t = cons.tile([128, 4 * D], F32R, tag="w1t")
    b1c = cons.tile([128, 4], F32, tag="b1c")
    nc.sync.dma_start(b1c[:], io["b1c"])
    ident = cons.tile([128, 128], F32, tag="ident")
    make_identity(nc, ident[:])
    ident_r = cons.tile([128, 128], F32R, tag="ident_r")
    nc.vector.tensor_copy(ident_r[:], ident[:])
    halfc = cons.tile([128, 1], F32, tag="halfc")
    nc.vector.memset(halfc[:], 0.5)
    if gemm_bias:
        bfg = cons.tile([64, 2048], BF16, tag="bfg")
        nc.sync.dma_start(bfg[0:1, :], io["bfr"])
        nc.sync.dma_start(bfg[32:33, :], io["bgr"])
        ones = cons.tile([64, 128], BF16, tag="ones")
        nc.vector.memset(ones[:], 1.0)
    # stage w1t fp32 -> fp32r via a closing pool
    with tc.tile_pool(name="init", bufs=1) as initp:
        w1t_f = initp.tile([128, 4 * D], F32, tag="w1t_f")
        for k in range(4):
            nc.sync.dma_start(w1t_f[:, k * 512:(k + 1) * 512],
                              io["w1t"][:, k * 512:(k + 1) * 512])
        nc.vector.tensor_copy(w1t[:], w1t_f[:])
    work = ctx.enter_context(tc.tile_pool(name="work", bufs=2))

    qtp = ctx.enter_context(tc.tile_pool(name="qtp", bufs=1))

    def make_qprep_parts(b, first):
        """Qmask + Q natural (fp32r) + Q_^T staging, as 4 queueable parts:
        [h0 load+transpose, h0 Q_ matmuls, h1 load+transpose, h1 matmuls]."""
        qc = {}
        hold = {}

        def load(h):
            def f():
                if "qma" not in qc:
                    qc["qma"] = batch.tile([128, LQ], BF16, tag="qma",
                                           name="qma")
                    qc["qb"] = batch.tile([128, 4 * D], F32R, tag="qb",
                                          name="qb")
                    qc["q_t"] = batch.tile([128, 4 * LQ], F32R, tag="q_t",
                                           name="q_t")
                qt = qtp.tile([128, 4 * 256], F32R, tag="qt", name="qt")
                hold["qt"] = qt
                for qi in range(2):
                    qc_i = 2 * h + qi
                    q_nat = work.tile([128, D], F32, tag="q_nat")
                    nc.sync.dma_start(
                        q_nat[:],
                        io["q_in"][b, qc_i * 128:(qc_i + 1) * 128, :])
                    if first:
                        drip_w(1)
                    if qc_i == 1:
                        nc.sync.dma_start(qc["qma"][:], io["qma"][b])
                    nc.vector.tensor_copy(
                        qc["qb"][:, qc_i * D:(qc_i + 1) * D], q_nat[:])
                    ps = psB.tile([128, 512], F32, tag="psB")
                    for k in range(4):
                        nc.tensor.transpose(ps[:, k * 128:(k + 1) * 128],
                                            q_nat[:, k * 128:(k + 1) * 128],
                                            ident[:])
                    dst = qt[:].rearrange("p (k q) -> p k q", k=4)[
                        :, :, qi * 128:(qi + 1) * 128]
                    nc.vector.tensor_copy(
                        dst, ps[:].rearrange("p (k q) -> p k q", k=4))
            return f

        def mm(h):
            def f():
                qt = hold["qt"]
                for dc in range(4):
                    ps = psA.tile([128, 256], F32, tag="psA")
                    for k in range(4):
                        nc.tensor.matmul(
                            ps[:],
                            w1t[:, k * D + dc * 128:k * D + dc * 128 + 128],
                            qt[:, k * 256:(k + 1) * 256],
                            start=(k == 0), stop=(k == 3))
                    tmp = chain.tile([128, 256], F32, tag="tmp")
                    nc.scalar.activation(tmp[:], ps[:], AF.Identity,
                                         bias=b1c[:, dc:dc + 1], scale=1.0)
                    nc.vector.scalar_tensor_tensor(
                        qc["q_t"][:, dc * LQ + h * 256:dc * LQ + h * 256 + 256],
                        tmp[:], 0.01, tmp[:], op0=ALU.mult, op1=ALU.max)
            return f

        return [load(0), mm(0), load(1), mm(1)], qc

    def emit_cnat_dmas(b, g, drip=0):
        tiles = []
        for t in range(groups_s[b][g]):
            ci = base_s[b][g] + t
            cp = cn_pool.tile([128, D], F32, tag="c_nat0")
            nc.sync.dma_start(cp[:], io["c_in"][b, ci * 128:(ci + 1) * 128, :])
            if drip:
                drip_w(drip)
            tiles.append(cp)
        return tiles

    def make_prologue_parts(b, g, cn_tiles):
        """Closures: [transpose half0, transpose half1, C_ dc01, C_ dc23]."""
        gs = groups_s[b][g]
        W = gs * 128
        st = {"W": W}
        h0 = (gs + 1) // 2

        def tr2(lo, hi):
            def f():
                if "ct" not in st:
                    st["ct"] = grp2.tile([128, 4 * W], F32R, tag="ct_grp",
                                         name="ct_grp")
                for t in range(lo, hi):
                    c_nat = cn_tiles[t]
                    ps = psB.tile([128, 512], F32, tag="psB")
                    for k in range(4):
                        nc.tensor.transpose(ps[:, k * 128:(k + 1) * 128],
                                            c_nat[:, k * 128:(k + 1) * 128],
                                            ident[:])
                    dst = st["ct"][:].rearrange("p (k c) -> p k c", k=4)[
                        :, :, t * 128:(t + 1) * 128]
                    nc.vector.tensor_copy(
                        dst, ps[:].rearrange("p (k q) -> p k q", k=4))
            return f

        def cmm(dcs):
            def f():
                if "cg" not in st:
                    st["cg"] = grp.tile([128, 4 * W], F32R, tag="cgrp",
                                        name="cgrp")
                for dc in dcs:
                    ps = psA.tile([128, W], F32, tag="psA")
                    for k in range(4):
                        nc.tensor.matmul(
                            ps[:],
                            w1t[:, k * D + dc * 128:k * D + dc * 128 + 128],
                            st["ct"][:, k * W:(k + 1) * W],
                            start=(k == 0), stop=(k == 3))
                    tmp = chain.tile([128, W], F32, tag="tmp")
                    nc.scalar.activation(tmp[:], ps[:], AF.Identity,
                                         bias=b1c[:, dc:dc + 1], scale=1.0)
                    nc.vector.scalar_tensor_tensor(
                        st["cg"][:, dc * W:(dc + 1) * W],
                        tmp[:], 0.01, tmp[:], op0=ALU.mult, op1=ALU.max)
            return f

        return [tr2(0, h0), tr2(h0, gs), cmm([0, 1]), cmm([2, 3])], st

    def make_chain_parts(b, ci, pro_st, qc, st):
        g = grp_of_s[b][ci]
        t = ci - base_s[b][g]
        W = groups_s[b][g] * 128

        def p0():  # S + masked softmax -> pb (fp32r, scaled by 1/sum)
            ps_s = psA.tile([128, 512], F32, tag="psA")
            for dc in range(4):
                nc.tensor.matmul(
                    ps_s[:],
                    pro_st["cg"][:, dc * W + t * 128:dc * W + t * 128 + 128],
                    qc["q_t"][:, dc * LQ:(dc + 1) * LQ],
                    start=(dc == 0), stop=(dc == 3))
            s1 = chain.tile([128, 512], F32, tag="s1")
            nc.vector.tensor_add(s1[:], ps_s[:], qc["qma"][:])
            negm = chain.tile([128, 1], F32, tag="negm")
            nc.vector.reduce_max(negm[:], s1[:], axis=AX.X, negate=True)
            p_f = chain.tile([128, 512], F32, tag="s2")
            ssum = chain.tile([128, 1], F32, tag="ssum")
            nc.scalar.activation(p_f[:], s1[:], AF.Exp, bias=negm[:],
                                 scale=1.0, accum_out=ssum[:])
            rec = chain.tile([128, 1], F32, tag="rec")
            nc.vector.reciprocal(rec[:], ssum[:])
            pb = chain.tile([128, 512], F32R, tag="pb")
            nc.scalar.activation(pb[:], p_f[:], AF.Copy, bias=0.0,
                                 scale=rec[:])
            st["pb"] = pb

        def p1():  # P^T
            ps_pt = psB.tile([128, 512], F32R, tag="psB")
            for qq in range(4):
                nc.tensor.transpose(ps_pt[:, qq * 128:(qq + 1) * 128],
                                    st["pb"][:, qq * 128:(qq + 1) * 128],
                                    ident_r[:])
            pt = chain.tile([128, 512], F32R, tag="s2")
            nc.vector.tensor_copy(pt[:], _f32(ps_pt[:]))
            st["pt"] = pt

        def p2():  # attn natural + blend C reload
            ps_an = psB.tile([128, 512], F32, tag="psB")
            for qq in range(4):
                nc.tensor.matmul(ps_an[:],
                                 st["pt"][:, qq * 128:(qq + 1) * 128],
                                 qc["qb"][:, qq * D:(qq + 1) * D],
                                 start=(qq == 0), stop=(qq == 3))
            attn = work.tile([128, 512], F32R, tag="attn")
            nc.vector.tensor_copy(attn[:], ps_an[:])
            st["attn"] = attn
            c_nat = work.tile([128, D], F32, tag="c_nat1")
            nc.sync.dma_start(c_nat[:],
                              io["c_in"][b, ci * 128:(ci + 1) * 128, :])
            st["c_nat"] = c_nat

        def p3():  # attn^T + T-layout cat blocks + natural cat blocks
            ps_at = psB.tile([128, 512], F32R, tag="psB")
            for dc in range(4):
                nc.tensor.transpose(ps_at[:, dc * 128:(dc + 1) * 128],
                                    st["attn"][:, dc * 128:(dc + 1) * 128],
                                    ident_r[:])
            attnt = work.tile([128, 512], BF16, tag="attnt")
            nc.vector.tensor_copy(attnt[:], _f32(ps_at[:]))
            ct_sl = _f32(pro_st["ct"][:]).rearrange("p (k c) -> p k c", k=4)[
                :, :, t * 128:(t + 1) * 128]
            multt = work.tile([128, 512], F8 if use_fp8 else BF16,
                              tag="multt")
            nc.vector.tensor_mul(
                multt[:].rearrange("p (k c) -> p k c", k=4),
                _f32(ps_at[:]).rearrange("p (k c) -> p k c", k=4), ct_sl)
            ctbf = work.tile([128, 512], BF16, tag="ctbf")
            nc.vector.tensor_copy(
                ctbf[:].rearrange("p (k c) -> p k c", k=4), ct_sl)
            amc = work.tile([128, 512], F32, tag="amc")
            nc.vector.tensor_sub(amc[:], _f32(st["attn"][:]), st["c_nat"][:])
            amm = work.tile([128, 512], F32, tag="amm")
            nc.vector.tensor_mul(amm[:], _f32(st["attn"][:]), st["c_nat"][:])
            st.update(attnt=attnt, multt=multt, ctbf=ctbf, amc=amc, amm=amm)

        return [p0, p1, p2, p3]

    def gemm_stage(bi, b, ci, ch_st, pend):
        key = (bi, ci)
        # everything this tile depends on must be emitted before its MMs
        while any(k == key for k, _ in pend):
            _, part = pend.popleft()
            part()
        emitted = 0
        for fc in range(4):
            while pend and emitted < fc + 2:
                _, part = pend.popleft()
                part()
                emitted += 1
            ps_f = psG.tile([128, 512], F32, tag="psG")
            ps_g = psG.tile([128, 512], F32, tag="psG")
            if gemm_bias:
                nc.tensor.matmul(ps_f[:], ones[0:1, :],
                                 bfg[0:1, fc * 512:(fc + 1) * 512],
                                 start=True, stop=False)
                nc.tensor.matmul(ps_g[:], ones[32:33, :],
                                 bfg[32:33, fc * 512:(fc + 1) * 512],
                                 start=True, stop=False)
            for jc in range(nbf):
                if jc < 4:
                    src = ch_st["ctbf"][:, (jc % 4) * 128:(jc % 4) * 128 + 128]
                elif jc < 8:
                    src = ch_st["attnt"][:, (jc % 4) * 128:(jc % 4) * 128 + 128]
                else:
                    src = ch_st["multt"][:, (jc % 4) * 128:(jc % 4) * 128 + 128]
                stt = (jc == 0) and not gemm_bias
                last = (jc == 11) and not use_fp8
                nc.tensor.matmul(ps_f[:], src,
                                 wf_ch[jc][:, fc * 512:fc * 512 + 512],
                                 start=stt, stop=last)
                nc.tensor.matmul(ps_g[:], src,
                                 wg_ch[jc][:, fc * 512:fc * 512 + 512],
                                 start=stt, stop=last)
            if use_fp8:
                m8 = ch_st["multt"][:].rearrange("p (k c) -> p k c", k=4)
                wf8_3 = wf8[:].rearrange("p (k f) -> p k f", k=4)
                wg8_3 = wg8[:].rearrange("p (k f) -> p k f", k=4)
                for j2 in range(2):
                    ksl = slice(2 * j2, 2 * j2 + 2)
                    nc.tensor.matmul(
                        ps_f[:], m8[:, ksl, :],
                        wf8_3[:, ksl, fc * 512:(fc + 1) * 512],
                        start=False, stop=(j2 == 1), perf_mode=DR)
                    nc.tensor.matmul(
                        ps_g[:], m8[:, ksl, :],
                        wg8_3[:, ksl, fc * 512:(fc + 1) * 512],
                        start=False, stop=(j2 == 1), perf_mode=DR)
            fuse = chain.tile([128, 512], F32, tag="fuse")
            nc.scalar.activation(fuse[:], ps_f[:], AF.Tanh, scale=1.0 / WSCALE)
            gth = chain.tile([128, 512], F32, tag="gth")
            nc.scalar.activation(gth[:], ps_g[:], AF.Tanh, scale=0.5 / WSCALE)
            gate = chain.tile([128, 512], F32, tag="gate")
            nc.scalar.activation(gate[:], gth[:], AF.Identity,
                                 bias=halfc[:], scale=0.5)
            cat_ap = [ch_st["c_nat"][:], _f32(ch_st["attn"][:]),
                      ch_st["amc"][:], ch_st["amm"][:]][fc]
            d1 = chain.tile([128, 512], F32, tag="d1")
            nc.vector.tensor_sub(d1[:], fuse[:], cat_ap)
            d2 = chain.tile([128, 512], F32, tag="fuse")
            nc.vector.tensor_mul(d2[:], d1[:], gate[:])
            out_t = work.tile([128, 512], F32, tag="out_t")
            nc.vector.tensor_add(out_t[:], d2[:], cat_ap)
            nc.sync.dma_start(
                io["out"][b, ci * 128:(ci + 1) * 128,
                          fc * 512:(fc + 1) * 512], out_t[:])

    seq = [bb for _ in range(repeat) for bb in range(nslots)]
    nseq = len(seq)
    pend = deque()
    qcs = {}           # bi -> qc state dict
    pro_states = {}    # (bi, g) -> prologue state
    cn_states = {}     # (bi, g) -> c_nat tiles
    ch_states = {}     # (bi, ci) -> chain state

    def queue_chain(bi, cc):
        stx = {}
        ch_states[(bi, cc)] = stx
        holder = {}
        b = seq[bi]

        def fmk(i):
            def f():
                if "parts" not in holder:
                    holder["parts"] = make_chain_parts(
                        b, cc, pro_states[(bi, grp_of_s[b][cc])],
                        qcs[bi], stx)
                holder["parts"][i]()
            return f
        for i in range(4):
            pend.append(((bi, cc), fmk(i)))

    def queue_prologue(bi, gg):
        holder = {}
        b = seq[bi]

        def fmk(i):
            def f():
                if "parts" not in holder:
                    holder["parts"], pst = make_prologue_parts(
                        b, gg, cn_states[(bi, gg)])
                    pro_states[(bi, gg)] = pst
                holder["parts"][i]()
            return f
        for i in range(4):
            pend.append(((bi, gg, 98), fmk(i)))

    def queue_cn(bi, gg):
        def f():
            cn_states[(bi, gg)] = emit_cnat_dmas(seq[bi], gg)
        pend.append(((bi, gg, 99), f))

    def queue_qprep(bi, idxs):
        def fmk(i):
            def f():
                if bi not in qcs:
                    parts, qc = make_qprep_parts(seq[bi], first=False)
                    qcs[bi] = qc
                    qcs[(bi, "parts")] = parts
                qcs[(bi, "parts")][i]()
            return f
        for i in idxs:
            pend.append(((bi, "q", i), fmk(i)))

    for bi, b in enumerate(seq):
        if bi == 0:
            qparts, qc0 = make_qprep_parts(b, first=True)
            qcs[0] = qc0
            for p in qparts:
                p()
            cn_states[(0, 0)] = emit_cnat_dmas(b, 0, drip=1)
            pro_parts, pro_st0 = make_prologue_parts(b, 0, cn_states[(0, 0)])
            for p in pro_parts:
                p()
            pro_states[(0, 0)] = pro_st0
        else:
            while pend:       # leftover prologue(bi,0)/cn parts, if any
                pend.popleft()[1]()
            if (bi, 0) not in pro_states:   # xslot=False: serial prologue
                cn_states[(bi, 0)] = emit_cnat_dmas(b, 0)
                pro_parts, pst = make_prologue_parts(b, 0, cn_states[(bi, 0)])
                for p in pro_parts:
                    p()
                pro_states[(bi, 0)] = pst
            if bi not in qcs:               # xslot<2: serial qprep
                parts, qc = make_qprep_parts(b, first=False)
                qcs[bi] = qc
                for p in parts:
                    p()
        if (bi, 0) not in ch_states:
            ch_st0 = {}
            ch_states[(bi, 0)] = ch_st0
            for p in make_chain_parts(b, 0, pro_states[(bi, 0)],
                                      qcs[bi], ch_st0):
                p()
        if bi == 0:
            drip_w(len(pending_w))

        nct = caps[b]
        for ci in range(nct):
            if ci + 1 < nct and (bi, ci + 1) not in ch_states:
                queue_chain(bi, ci + 1)
            for g2 in range(grp_of_s[b][ci] + 1, ngrp_s[b]):
                if ci == base_s[b][g2] - 3:
                    queue_cn(bi, g2)
                elif ci == base_s[b][g2] - 2:
                    queue_prologue(bi, g2)
            if xslot and bi + 1 < nseq:
                if ci == max(0, nct - 3):
                    queue_cn(bi + 1, 0)
                if ci == max(0, nct - 2):
                    queue_prologue(bi + 1, 0)
                    if xslot == 2:
                        queue_qprep(bi + 1, [0, 1])
                if ci == nct - 1 and xslot == 2:
                    queue_qprep(bi + 1, [2, 3])
                    queue_chain(bi + 1, 0)
            gemm_stage(bi, b, ci, ch_states[(bi, ci)], pend)
            ch_states.pop((bi, ci), None)
        # drop per-slot state we no longer need
        if bi >= 1:
            qcs.pop(bi - 1, None)
            qcs.pop((bi - 1, "parts"), None)
            for g in range(max(ngrp_s)):
                pro_states.pop((bi - 1, g), None)
                cn_states.pop((bi - 1, g), None)
    while pend:
        pend.popleft()[1]()


_CACHE = {}


def _get_module(caps, gemm_bias, repeat=1, xslot=2, use_fp8=True):
    caps = tuple(caps)
    key = (caps, gemm_bias, repeat, xslot, use_fp8)
    if key in _CACHE:
        return _CACHE[key]
    nc = bacc.Bacc("TRN2", target_bir_lowering=False, debug=False,
                   num_devices=N_CORES)
    lc = max(caps) * 128
    io = {
        "c_in": nc.dram_tensor("c_in", [len(caps), lc, D], F32,
                               kind="ExternalInput").ap(),
        "q_in": nc.dram_tensor("q_in", [NSLOTS, LQ, D], F32,
                               kind="ExternalInput").ap(),
        "w1t": nc.dram_tensor("w1t", [128, 4 * D], F32,
                              kind="ExternalInput").ap(),
        "wft": nc.dram_tensor("wft", [128, 12 * 2048], BF16,
                              kind="ExternalInput").ap(),
        "wgt": nc.dram_tensor("wgt", [128, 12 * 2048], BF16,
                              kind="ExternalInput").ap(),
        "wf8": nc.dram_tensor("wf8", [128, 4 * 2048], F8,
                              kind="ExternalInput").ap(),
        "wg8": nc.dram_tensor("wg8", [128, 4 * 2048], F8,
                              kind="ExternalInput").ap(),
        "b1c": nc.dram_tensor("b1c", [128, 4], F32, kind="ExternalInput").ap(),
        "bfr": nc.dram_tensor("bfr", [1, 2048], BF16,
                              kind="ExternalInput").ap(),
        "bgr": nc.dram_tensor("bgr", [1, 2048], BF16,
                              kind="ExternalInput").ap(),
        "qma": nc.dram_tensor("qma", [NSLOTS, 128, LQ], BF16,
                              kind="ExternalInput").ap(),
        "out": nc.dram_tensor("out", [NSLOTS, lc, 4 * D], F32,
                              kind="ExternalOutput").ap(),
    }
    with tile.TileContext(nc) as tc, ExitStack() as ctx:
        _body(ctx, tc, io, caps, gemm_bias, repeat, xslot, use_fp8)
    nc.compile()
    _CACHE[key] = nc
    return nc


def _weff(W):
    """Folded GEMM weights, pre-scaled by WSCALE: bf16 part (C and A
    blocks, 8 k-chunks) and fp8 part (A*C block, 4 k-chunks)."""
    b0, b1_, b2, b3 = (W[:, i * 512:(i + 1) * 512] for i in range(4))
    weff = np.concatenate([b0 - b2, b1_ + b2, b3], axis=1)  # [2048, 1536]
    wt = np.ascontiguousarray(weff.T) * np.float32(WSCALE)  # [1536, 2048]
    wbf = np.ascontiguousarray(
        wt.reshape(12, 128, 2048).transpose(1, 0, 2)
        .reshape(128, 12 * 2048)).astype(BF16_NP)
    w8 = np.ascontiguousarray(
        wt[1024:].reshape(4, 128, 2048).transpose(1, 0, 2)
        .reshape(128, 4 * 2048)).astype(F8_NP)
    return wbf, w8


def _try_pack(t_desc, cap_list):
    """Greedily pack per-batch tile counts (desc order) into slots with the
    given capacities (desc order); each slot holds tiles of one batch only.
    Returns assignment [(batch, ntiles)] per slot, or None if infeasible."""
    out = [None] * len(cap_list)
    free = list(range(len(cap_list)))
    for bidx, tb in t_desc:
        rem = tb
        while rem > 0:
            if not free:
                return None
            s = free.pop(0)          # largest remaining capacity first
            take = min(rem, cap_list[s])
            if take == 0:
                continue
            out[s] = (bidx, take)
            rem -= take
    return out


def _plan(Cmask):
    """Pack unmasked rows into NSLOT_TOTAL single-batch slots. Each core
    runs NSLOTS slots with capacities caps (same across cores, possibly
    unequal between slot 0 and 1) -- minimizes sum(caps) = tiles/core.

    Returns (caps, slots): caps = per-core-slot tile capacities; slots =
    list of (batch, row_index_array), slot i*NSLOTS+sl has capacity
    caps[sl]; padded with empties.
    """
    B = Cmask.shape[0]
    idx = [np.flatnonzero(Cmask[b]) for b in range(B)]
    t = [-(-len(ix) // 128) for ix in idx]
    t_desc = sorted(((b, tb) for b, tb in enumerate(t) if tb),
                    key=lambda x: -x[1])
    total = sum(t)
    best = None
    for c0 in range(1, 17):
        for c1 in range(0, c0 + 1):
            if N_CORES * (c0 + c1) < total:
                continue
            cap_list = [c0] * N_CORES + [c1] * N_CORES
            asn = _try_pack(t_desc, cap_list)
            if asn is not None:
                cand = (c0 + c1, c0, (c0, c1), asn)
                if best is None or cand[:2] < best[:2]:
                    best = cand
    assert best is not None
    (c0, c1), asn = best[2], best[3]
    # slot order in asn: 8x capacity c0 then 8x capacity c1; interleave to
    # (core0 slot0, core0 slot1, ...) = capacities (c0, c1) per core.
    used = {}
    slots = []
    for i in range(N_CORES):
        for sl, s_glob in ((0, i), (1, N_CORES + i)):
            a = asn[s_glob]
            if a is None:
                slots.append((0, np.zeros(0, np.int64)))
            else:
                bidx, take = a
                lo = used.get(bidx, 0)
                used[bidx] = lo + take
                slots.append((bidx, idx[bidx][lo * 128:(lo + take) * 128]))
    return (c0, max(c1, 1)), slots


def host_inputs(C, Q, Qmask, W1, b1, Wf, bf, Wg, bg, caps, slots):
    cap = max(caps) * 128
    w1t = np.ascontiguousarray(
        np.ascontiguousarray(W1.T).reshape(4, 128, D)
        .transpose(1, 0, 2).reshape(128, 4 * D), dtype=np.float32)
    wft, wf8 = _weff(Wf)
    wgt, wg8 = _weff(Wg)
    b1c = np.ascontiguousarray(b1.reshape(4, 128).T, dtype=np.float32)
    bfr = np.ascontiguousarray(bf.reshape(1, 2048) * WSCALE,
                               dtype=np.float32).astype(BF16_NP)
    bgr = np.ascontiguousarray(bg.reshape(1, 2048) * WSCALE,
                               dtype=np.float32).astype(BF16_NP)
    qm_pen = ((Qmask.astype(np.float32) - np.float32(1.0))
              * np.float32(1e30))                       # [B, LQ]
    maps = []
    for i in range(N_CORES):
        c_in = np.zeros((NSLOTS, cap, D), np.float32)
        q_in = np.zeros((NSLOTS, LQ, D), np.float32)
        qma = np.zeros((NSLOTS, 128, LQ), BF16_NP)
        for sl in range(NSLOTS):
            bsl, rows = slots[i * NSLOTS + sl]
            if len(rows):
                c_in[sl, :len(rows)] = C[bsl][rows]
                q_in[sl] = Q[bsl]
                qma[sl] = np.broadcast_to(qm_pen[bsl][None, :],
                                          (128, LQ)).astype(BF16_NP)
        maps.append({
            "c_in": c_in, "q_in": q_in,
            "w1t": w1t, "wft": wft, "wgt": wgt, "wf8": wf8, "wg8": wg8,
            "b1c": b1c, "bfr": bfr, "bgr": bgr, "qma": qma,
        })
    return maps


def kernel(C, Q, Cmask, Qmask, W1, b1, Wf, bf, Wg, bg, _trace=False):
    C = np.asarray(C, dtype=np.float32)
    Q = np.asarray(Q, dtype=np.float32)
    Cmask = np.asarray(Cmask)
    Qmask = np.asarray(Qmask)
    W1 = np.asarray(W1, dtype=np.float32)
    b1 = np.asarray(b1, dtype=np.float32)
    Wf = np.asarray(Wf, dtype=np.float32)
    bf = np.asarray(bf, dtype=np.float32)
    Wg = np.asarray(Wg, dtype=np.float32)
    bg = np.asarray(bg, dtype=np.float32)

    B, Lc = Cmask.shape
    out_full = np.full((B, Lc, 4 * D), np.float32(-1e30), np.float32)
    if not Cmask.any():
        return out_full

    caps, slots = _plan(Cmask)
    gemm_bias = bool(np.any(bf) or np.any(bg))
    nc = _get_module(caps, gemm_bias)
    maps = host_inputs(C, Q, Qmask, W1, b1, Wf, bf, Wg, bg, caps, slots)
    res = run_bass_kernel_spmd(nc, maps, list(range(N_CORES)), trace=_trace)
    for s, (bsl, rows) in enumerate(slots):
        if len(rows):
            core, sl = divmod(s, NSLOTS)
            out_full[bsl, rows, :] = res.results[core]["out"][sl, :len(rows)]
    if _trace:
        return out_full, res
    return out_full
